# revision 17
# baseline (speedup 1.0000x reference)
"""Radon transform (bilinear grid-sample + row-sum) on 8 TRN2 NeuronCores.

Strategy: angle wedges are sharded across the 8 cores (rep-pure wedges: cores
0-3 process angles where |cos|>=|sin| on the identity frame, cores 4-7 the
rest on the transposed frame). On each core the 4 bilinear taps x 2 batches of
every sample are fetched with a single shared GPSIMD ap_gather index
(channel-shifted slab copies per 16-partition group; d=2 gathers the
horizontal tap pair; column parity handled by zeroing weights on wrong-parity
channels). Weighted taps are combined by DVE multiply + segment-reduce and a
TensorE ones-matmul partition-sum produces each sinogram column.

All gather indices / weights are input-independent and precomputed on host.
"""
import math
import os
import sys
from contextlib import ExitStack

import numpy as np

sys.path.insert(0, "/opt/trn_rl_repo")

import ml_dtypes  # noqa: E402

BF16 = ml_dtypes.bfloat16

# ─── geometry constants (hardcoded for 256x256, 180 angles, batch 2) ───
N_ANGLES = 180
IMG_SIZE = 256
BATCH = 2
S = int(math.ceil(math.sqrt(2.0) * IMG_SIZE))  # 363
PB = (S - IMG_SIZE) // 2                       # 53
FR = 520          # embedded frame size
EMB = 78          # embedding offset: tap rows/cols land in [3, 514]
NJ = 64           # j-rows per band-class slab
NBLK = 256        # d=2 blocks per j-row
WID = 512
NELEM_BLK = NJ * NBLK  # 16384 gather blocks per frame
NGROUP = 8
SLOTS = 23
SXPAD = 368       # 16*23, padded xg dimension
CHUNK_NXG = [80, 80, 80, 80, 48]

CORE_ANGLES = [
    list(range(0, 23)), list(range(23, 46)),
    list(range(135, 158)), list(range(158, 180)),
    list(range(46, 69)), list(range(69, 91)),
    list(range(91, 113)), list(range(113, 135)),
]
CORE_REP = [0, 0, 0, 0, 1, 1, 1, 1]


def _angle_tables(k):
    theta = np.float32(k) * np.float32(np.pi / N_ANGLES)
    c = np.cos(theta, dtype=np.float32)
    s = np.sin(theta, dtype=np.float32)
    lin = np.linspace(-1.0, 1.0, S, dtype=np.float32)
    x = lin[None, :]
    y = lin[:, None]
    gx = c * x + s * y
    gy = -s * x + c * y
    ix = ((gx + np.float32(1.0)) * np.float32(0.5) * np.float32(S - 1)).astype(np.float32)
    iy = ((gy + np.float32(1.0)) * np.float32(0.5) * np.float32(S - 1)).astype(np.float32)
    x0 = np.floor(ix)
    y0 = np.floor(iy)
    wx = ix - x0
    wy = iy - y0
    return (y0.astype(np.int32), x0.astype(np.int32),
            (1 - wx) * (1 - wy), wx * (1 - wy), (1 - wx) * wy, wx * wy, c, s)


def _plan_angle(k, lseg):
    """idx [8, nidx] int32 block indices, wch [8, 4, nidx, 2] f32 weights."""
    y0, x0, w00, w01, w10, w11, c, s = _angle_tables(k)
    y0e, x0e = y0 + EMB, x0 + EMB
    rep = 0 if abs(c) >= abs(s) else 1
    if rep == 0:
        r0, c0 = y0e, x0e
        wp = (w00, w01, w10, w11)  # cls = sr*2+par ; pair elem = dx
    else:
        r0, c0 = x0e, y0e
        wp = (w00, w10, w01, w11)  # sr = dx ; pair elem = dy
    gp = (r0 - 3) % 8
    j = (r0 - 3) // 8
    lx = c0 - 3
    par = lx % 2
    idx_flat = j * NBLK + lx // 2
    nidx = SXPAD * lseg
    idx = np.zeros((NGROUP, nidx), np.int32)
    wch = np.zeros((NGROUP, 4, nidx, 2), np.float32)
    for g in range(NGROUP):
        ygs, xgs = np.nonzero(gp == g)
        order = np.argsort(xgs, kind="stable")
        ygs, xgs = ygs[order], xgs[order]
        cnt = np.bincount(xgs, minlength=S)
        assert cnt.max() <= lseg, (k, g, cnt.max(), lseg)
        starts = np.concatenate([[0], np.cumsum(cnt)[:-1]])
        t = np.arange(len(xgs)) - starts[xgs]
        slot = xgs * lseg + t
        idx[g, slot] = idx_flat[ygs, xgs]
        pr = par[ygs, xgs]
        for sr in range(2):
            # pair elem e: weight of (sr, e): rep0: w[dy=sr][dx=e]; rep1: w[dy=e][dx=sr]
            we0 = wp[sr * 2 + 0][ygs, xgs]
            we1 = wp[sr * 2 + 1][ygs, xgs]
            for pp in range(2):
                cls = sr * 2 + pp
                m = (pr == pp).astype(np.float32)
                wch[g, cls, slot, 0] = we0 * m
                wch[g, cls, slot, 1] = we1 * m
    return rep, idx, wch


def _slot_lsegs():
    ls = np.zeros(SLOTS, np.int64)
    for ci in range(8):
        for si, k in enumerate(CORE_ANGLES[ci]):
            y0, x0, _, _, _, _, c, s = _angle_tables(k)
            rep = 0 if abs(c) >= abs(s) else 1
            r0 = (y0 if rep == 0 else x0) + EMB
            gp = (r0 - 3) % 8
            m = 0
            for g in range(NGROUP):
                m = max(m, int(np.bincount(np.nonzero(gp == g)[1], minlength=S).max()))
            ls[si] = max(ls[si], m)
    # make nidx = SXPAD*lseg multiple of 16 (SXPAD=368 = 16*23 -> always ok)
    return [int(v) for v in ls]


_PLAN_CACHE = {}


def _get_plan():
    if "plan" in _PLAN_CACHE:
        return _PLAN_CACHE["plan"]
    lsegs = _slot_lsegs()
    nidxs = [SXPAD * L for L in lsegs]
    # per-core packed idx blobs ([128, tot16] int16) and weight blobs
    # ([32, totw] bf16), plus chunk offset tables
    tot16 = sum(n // 16 for n in nidxs)
    totw = sum(n * 2 for n in nidxs)
    core_idx = []
    core_w = []
    for ci in range(8):
        idx_blob = np.zeros((128, tot16), np.int16)
        w_blob = np.zeros((32, totw), np.float32)
        o16 = 0
        ow = 0
        for si, k in enumerate(CORE_ANGLES[ci]):
            L = lsegs[si]
            n = nidxs[si]
            rep, idx, wch = _plan_angle(k, L)
            assert rep == CORE_REP[ci]
            for g in range(NGROUP):
                wrap = idx[g].reshape(n // 16, 16).T.astype(np.int16)  # [16, n/16]
                idx_blob[16 * g:16 * g + 16, o16:o16 + n // 16] = wrap
                for cls in range(4):
                    w_blob[g * 4 + cls, ow:ow + 2 * n] = wch[g, cls].reshape(-1)
            o16 += n // 16
            ow += 2 * n
        core_idx.append(idx_blob)
        core_w.append(w_blob.astype(BF16))
    sel = np.zeros((128, 2), np.float32)
    for p in range(128):
        cch = p % 16
        if cch % 2 == 0:
            sel[p, (cch % 4) // 2] = 1.0
    plan = dict(lsegs=lsegs, nidxs=nidxs, tot16=tot16, totw=totw,
                core_idx=core_idx, core_w=core_w, sel=sel)
    _PLAN_CACHE["plan"] = plan
    return plan


def _build_frame(image, rep):
    fr = np.zeros((BATCH, FR, FR), np.float32)
    img_s = np.zeros((BATCH, S, S), np.float32)
    img_s[:, PB:PB + IMG_SIZE, PB:PB + IMG_SIZE] = image[:, 0]
    fr[:, EMB:EMB + S, EMB:EMB + S] = img_s
    if rep:
        fr = np.ascontiguousarray(np.transpose(fr, (0, 2, 1)))
    return fr


def _build_slabs(frame):
    out = np.zeros((128, NELEM_BLK * 2), np.float32)
    for gp in range(NGROUP):
        for sr in range(2):
            rows = frame[:, 3 + gp + sr: 3 + gp + sr + 8 * NJ: 8, :]  # [B,64,520]
            for par in range(2):
                cols = rows[:, :, 3 + par: 3 + par + 2 * NBLK]  # [B,64,512]
                flat = cols.reshape(BATCH, -1)
                for b in range(BATCH):
                    for dup in range(2):
                        p = 16 * gp + (sr * 2 + par) * 4 + b * 2 + dup
                        out[p] = flat[b]
    return out


_PROG_CACHE = {}


def _build_program(plan):
    if "prog" in _PROG_CACHE:
        return _PROG_CACHE["prog"]
    import concourse.bass as bass
    import concourse.mybir as mybir
    from concourse import library_config

    lsegs = plan["lsegs"]
    nidxs = plan["nidxs"]
    maxcn = max(CHUNK_NXG) * max(lsegs)

    nc = bass.Bass()
    slab_d = nc.declare_dram_parameter("slab", [128, NELEM_BLK * 2],
                                       mybir.dt.bfloat16, isOutput=False)
    idx_d = nc.declare_dram_parameter("idx", [128, plan["tot16"]],
                                      mybir.dt.int16, isOutput=False)
    w_d = nc.declare_dram_parameter("w", [32, plan["totw"]],
                                    mybir.dt.bfloat16, isOutput=False)
    sel_d = nc.declare_dram_parameter("sel", [128, 2], mybir.dt.float32,
                                      isOutput=False)
    out_d = nc.declare_dram_parameter("out", [SLOTS, 2, SXPAD],
                                      mybir.dt.float32, isOutput=True)
    debug = bool(os.environ.get("RADON_DEBUG"))
    if debug:
        maxcn0 = max(CHUNK_NXG) * max(plan["lsegs"])
        dbg_g = nc.declare_dram_parameter("dbg_g", [128, maxcn0 * 2],
                                          mybir.dt.bfloat16, isOutput=True)
        dbg_w = nc.declare_dram_parameter("dbg_w", [128, maxcn0 * 2],
                                          mybir.dt.bfloat16, isOutput=True)
        dbg_p = nc.declare_dram_parameter("dbg_p", [128, maxcn0 * 2],
                                          mybir.dt.bfloat16, isOutput=True)
        dbg_r = nc.declare_dram_parameter("dbg_r", [128, SXPAD],
                                          mybir.dt.float32, isOutput=True)

    ctx = ExitStack()
    with ctx:
        slab_t = ctx.enter_context(nc.sbuf_tensor([128, NELEM_BLK * 2], mybir.dt.bfloat16))
        idx_t = ctx.enter_context(nc.sbuf_tensor([128, maxcn // 16], mybir.dt.int16))
        w_t = ctx.enter_context(nc.sbuf_tensor([128, maxcn * 2], mybir.dt.bfloat16))
        g_t = ctx.enter_context(nc.sbuf_tensor([128, maxcn * 2], mybir.dt.bfloat16))
        p_t = ctx.enter_context(nc.sbuf_tensor([128, maxcn * 2], mybir.dt.bfloat16))
        r_ts = [ctx.enter_context(nc.sbuf_tensor(f"r{i}", [128, SXPAD], mybir.dt.float32)) for i in range(2)]
        sel_t = ctx.enter_context(nc.sbuf_tensor([128, 2], mybir.dt.float32))
        vscr_t = ctx.enter_context(nc.sbuf_tensor([128, 2], mybir.dt.float32))
        ascr_t = ctx.enter_context(nc.sbuf_tensor([2, 2], mybir.dt.float32))
        sino_t = ctx.enter_context(nc.sbuf_tensor("sino", [2, SLOTS * SXPAD],
                                                   mybir.dt.float32))
        psum_ts = [ctx.enter_context(nc.psum_tensor(f"ps{i}", [2, SXPAD], mybir.dt.float32)) for i in range(2)]
        s_in = ctx.enter_context(nc.semaphore("s_in"))
        s_dma = ctx.enter_context(nc.semaphore("s_dma"))
        s_g = ctx.enter_context(nc.semaphore("s_g"))
        s_v = ctx.enter_context(nc.semaphore("s_v"))
        s_mm = ctx.enter_context(nc.semaphore("s_mm"))
        s_cp = ctx.enter_context(nc.semaphore("s_cp"))
        s_od = ctx.enter_context(nc.semaphore("s_od"))
        s_dbg = ctx.enter_context(nc.semaphore("s_dbg"))
        block = ctx.enter_context(nc.Block())

        # chunk schedule: list of (slot, ci, xoff, nxg, cn, o16, ow)
        chunks = []
        o16 = ow = 0
        for si in range(SLOTS):
            L = lsegs[si]
            xoff = 0
            for cidx, nxg in enumerate(CHUNK_NXG):
                cn = nxg * L
                chunks.append(dict(si=si, cidx=cidx, L=L, xoff=xoff, nxg=nxg,
                                   cn=cn, o16=o16, ow=ow))
                xoff += nxg
                o16 += cn // 16
                ow += 2 * cn
        nchunks = len(chunks)

        @block.sync
        def _(sync):
            sync.dma_start(out=slab_t[:], in_=slab_d[:]).then_inc(s_in, 16)
            sync.dma_start(out=sel_t[:], in_=sel_d[:]).then_inc(s_in, 16)
            for n, ch in enumerate(chunks):
                # reuse guards: idx_t read by gather n-1; w_t read by vector n-1
                if n > 0:
                    sync.wait_ge(s_g, n)
                    sync.wait_ge(s_v, n)
                if debug and n == len(CHUNK_NXG):
                    sync.wait_ge(s_v, len(CHUNK_NXG))
                    sync.dma_start(out=dbg_r[:], in_=r_ts[0][:]).then_inc(s_dbg, 16)
                sync.dma_start(
                    out=idx_t[:, :ch["cn"] // 16],
                    in_=idx_d[:, ch["o16"]:ch["o16"] + ch["cn"] // 16],
                ).then_inc(s_dma, 16)
                wsrc = (w_d[:, ch["ow"]:ch["ow"] + 2 * ch["cn"]]
                        .unsqueeze(1).broadcast_to([32, 4, 2 * ch["cn"]]))
                sync.dma_start(out=w_t[:, :2 * ch["cn"]], in_=wsrc).then_inc(s_dma, 16)

        @block.gpsimd
        def _(g):
            g.load_library(library_config.ap_gather)
            g.wait_ge(s_in, 32)
            # warmup + startup barrier with VALID indices (chunk 0 already
            # DMA'd): burn ~200us so all preamble DMA descriptor streams
            # (slab) have fully landed before the real gathers
            g.wait_ge(s_dma, 32)
            ch0 = chunks[0]
            for _ in range(2):
                g.ap_gather(
                    g_t[:, :2 * ch0["cn"]].rearrange("p (n d) -> p n d", d=2),
                    slab_t[:].rearrange("p (n d) -> p n d", d=2),
                    idx_t[:, :ch0["cn"] // 16],
                    channels=128, num_elems=NELEM_BLK, d=2, num_idxs=ch0["cn"],
                )
            for n, ch in enumerate(chunks):
                g.wait_ge(s_dma, 32 * (n + 1))
                if n > 0:
                    g.wait_ge(s_v, n)  # g_t consumed by vector of chunk n-1
                g.ap_gather(
                    g_t[:, :2 * ch["cn"]].rearrange("p (n d) -> p n d", d=2),
                    slab_t[:].rearrange("p (n d) -> p n d", d=2),
                    idx_t[:, :ch["cn"] // 16],
                    channels=128, num_elems=NELEM_BLK, d=2, num_idxs=ch["cn"],
                ).then_inc(s_g, 1)

        @block.vector
        def _(v):
            for n, ch in enumerate(chunks):
                v.wait_ge(s_g, n + 1)
                if ch["cidx"] == 0 and ch["si"] > 1:
                    v.wait_ge(s_mm, ch["si"] - 1)  # r_t buffer consumed by matmul
                if debug and ch["si"] == 2 and ch["cidx"] == 0:
                    v.wait_ge(s_dbg, 16)  # r_ts[0] dumped before slot-2 overwrites
                v.tensor_mul(p_t[:, :2 * ch["cn"]], g_t[:, :2 * ch["cn"]],
                             w_t[:, :2 * ch["cn"]])
                rdst = r_ts[ch["si"] % 2]
                v.tensor_reduce(
                    out=rdst[:, ch["xoff"]:ch["xoff"] + ch["nxg"]],
                    in_=p_t[:, :2 * ch["cn"]].rearrange(
                        "p (x l) -> p x l", l=2 * ch["L"]),
                    axis=mybir.AxisListType.X,
                    op=mybir.AluOpType.add,
                )
                # drain fence: DVE issues in order after pipe empties, so this
                # inc observes the reduce's writes as complete
                v.tensor_copy(vscr_t[:, :1],
                              rdst[:, ch["xoff"]:ch["xoff"] + 1]).then_inc(s_v, 1)

        @block.tensor
        def _(t):
            for si in range(SLOTS):
                t.wait_ge(s_v, (si + 1) * len(CHUNK_NXG))
                if si > 1:
                    t.wait_ge(s_cp, si - 1)  # psum buffer consumed by scalar copy
                t.matmul(psum_ts[si % 2][:], sel_t[:], r_ts[si % 2][:],
                         start=True, stop=True).then_inc(s_mm, 1)

        @block.scalar
        def _(sc):
            for si in range(SLOTS):
                sc.wait_ge(s_mm, si + 1)
                sc.copy(sino_t[:, si * SXPAD:(si + 1) * SXPAD], psum_ts[si % 2][:])
                sc.copy(ascr_t[:, :1],
                        sino_t[:, si * SXPAD:si * SXPAD + 1]).then_inc(s_cp, 1)
            sc.wait_ge(s_cp, SLOTS)
            sc.dma_start(out=out_d.rearrange("s b x -> b s x"),
                         in_=sino_t[:].rearrange("b (s x) -> b s x", x=SXPAD)
                         ).then_inc(s_od, 16)
            sc.wait_ge(s_od, 16)

    mybir.codegen_inst_isa_subclasses(nc)
    _PROG_CACHE["prog"] = nc
    return nc


def kernel(image):
    image = np.asarray(image, np.float32)
    assert image.shape == (BATCH, 1, IMG_SIZE, IMG_SIZE)
    plan = _get_plan()
    nc = _build_program(plan)

    from concourse.bass_utils import run_bass_kernel_spmd

    in_maps = []
    for ci in range(8):
        frame = _build_frame(image, CORE_REP[ci])
        slab = _build_slabs(frame).astype(BF16)
        in_maps.append({
            "slab": slab,
            "idx": plan["core_idx"][ci],
            "w": plan["core_w"][ci],
            "sel": plan["sel"],
        })

    trace = bool(os.environ.get("RADON_TRACE"))
    if trace:
        _install_profhook()
    res = run_bass_kernel_spmd(nc, in_maps, list(range(8)), trace=trace)
    if trace:
        kernel.last_exec_time_ns = res.exec_time_ns

    sino = np.zeros((BATCH, 1, S, N_ANGLES), np.float32)
    for ci in range(8):
        o = res.results[ci]["out"]  # [SLOTS, 2, SXPAD]
        for si, k in enumerate(CORE_ANGLES[ci]):
            sino[:, 0, :, k] = o[si, :, :S]
    return sino


def _install_profhook():
    import types
    if "antenv.axon_hooks" in sys.modules:
        return
    try:
        from trn_agent_boot.trn_boot import _ntff_profile_via_ctypes
        hook = _ntff_profile_via_ctypes("/opt/axon/libaxon_pjrt.so")
    except Exception:
        hook = None
    mod = types.ModuleType("antenv.axon_hooks")
    mod._hook = hook
    mod.set_axon_ntff_profile_hook = lambda h: setattr(mod, "_hook", h)
    mod.get_axon_ntff_profile_hook = lambda: mod._hook
    sys.modules["antenv.axon_hooks"] = mod
    import antenv
    antenv.axon_hooks = mod


if __name__ == "__main__":
    img = np.load("/tmp/ref_image.npy")
    out = kernel(image=img)
    exp = np.load("/tmp/ref_expected.npy")
    err = np.linalg.norm(out - exp) / np.linalg.norm(exp)
    print("kernel rel err:", err)
